# revision 28
# baseline (speedup 1.0000x reference)
"""Linear attention kernel for 8 Trainium2 NeuronCores.

Sharding: core = 2*b + hg  (b in 0..3 batches, hg in 0..1 head-groups of 8 heads).
Fully data-parallel - no collectives; host sums the two head-group partials per
batch. Each core adds bias/2 so the pair-sum carries the full bias.

Per-core math (T=4096 tokens, CH=512 = 8 heads x 64, DIM=1024):
  qT   = (x @ Wq)^T          c-major [CH, T], elu+1
  k,v  = x @ Wk, x @ Wv      token-major [T, CH], elu+1 on k
  kvT  = v^T k  (per head-pair, diagonal 64-blocks valid), accumulated in PSUM
  z    = ones^T k            [1, CH], accumulated in PSUM
  den  = Zblock^T qT         [8, T]   (Zblock = block-diag scatter of z)
  r    = 1/den;  rbc = E^T r (broadcast r over each head's 64 rows)
  qsc  = qT * rbc
  M    = kvT^T @ W2  (per 128-row ch-tile; off-diag blocks of kvT zeroed)
  y    = qsc^T @ M + bias/2  token-major [T, DIM]

v2 scheduling: batched 3D-AP DMAs ordered by first use, PE warmup matmuls
during the DMA ramp (HAM stays warm), 3-op elu+1 (min/exp/stt-max), z
transposed to columns via K=1 matmuls at the phase boundary (no SB->SB DMA),
and a fused phase B with 2-stage lookahead so den/bc/recip of ib+1/ib+2
overlap the y matmuls of ib and the y writeback DMA is spread evenly.
"""

import sys

sys.path.insert(0, "/opt/trn_rl_repo")

import numpy as np

import concourse.bass as bass
import concourse.mybir as mybir
import concourse.tile as tile
from concourse import bacc

F32 = mybir.dt.float32
BF16 = mybir.dt.bfloat16
AF = mybir.ActivationFunctionType
ALU = mybir.AluOpType

DIM = 1024      # model dim (contraction for projections)
CH = 512        # per-core channels (8 heads x 64)
P = 128

N_CORES = 8
B, T_FULL = 4, 4096

WARMUP_MMS = 12


def build_nc(T=T_FULL):
    NTB = T // 512          # 512-token blocks
    nc = bacc.Bacc(None, target_bir_lowering=False, debug=False)

    # All inputs host-repacked partition-major so every DMA line is a
    # contiguous 2-8KB row segment (packet-count, not bandwidth, limits
    # the startup ramp).
    xP = nc.declare_dram_parameter("xP", [P, NTB * 8 * 512], BF16, isOutput=False)
    w1q = nc.declare_dram_parameter("w1q", [P, 4 * 1024], BF16, isOutput=False)
    w1k = nc.declare_dram_parameter("w1k", [P, 8 * 512], BF16, isOutput=False)
    w1v = nc.declare_dram_parameter("w1v", [P, 8 * 512], BF16, isOutput=False)
    w2 = nc.declare_dram_parameter("w2", [P, 4 * 1024], BF16, isOutput=False)
    ec = nc.declare_dram_parameter("ec", [8, CH], BF16, isOutput=False)
    y = nc.declare_dram_parameter("y", [T, DIM], F32, isOutput=True)

    with tile.TileContext(nc) as tc:
        with tc.tile_pool(name="persist", bufs=1) as pp:
            # ---- persistent tiles ----
            scratch = pp.tile([P, 512], BF16, name="scratch", tag="scratch")
            ones_col = pp.tile([P, 1], BF16, name="ones_col", tag="ones_col")
            w1qsb = pp.tile([P, 4, DIM], BF16, name="w1qsb", tag="w1qsb")
            w1ksb = pp.tile([P, 8, CH], BF16, name="w1ksb", tag="w1ksb")
            w1vsb = pp.tile([P, 8, CH], BF16, name="w1vsb", tag="w1vsb")
            w2sb = pp.tile([P, 4, DIM], BF16, name="w2sb", tag="w2sb")
            ec_sb = pp.tile([8, CH], BF16, name="ec_sb", tag="ec_sb")
            xsb = [
                pp.tile([P, 8, 512], BF16, name=f"xsb_{ib}", tag=f"xsb_{ib}")
                for ib in range(NTB)
            ]
            qt = [
                pp.tile([P, T], BF16, name=f"qt_{j}", tag=f"qt_{j}")
                for j in range(4)
            ]
            kvt = [
                pp.tile([P, P], BF16, name=f"kvt_{j}", tag=f"kvt_{j}")
                for j in range(4)
            ]
            Zb = [
                pp.tile([P, 8], BF16, name=f"Zb_{j}", tag=f"Zb_{j}")
                for j in range(4)
            ]
            Ms = [
                pp.tile([P, DIM], BF16, name=f"Ms_{j}", tag=f"Ms_{j}")
                for j in range(4)
            ]
            zt = pp.tile([1, CH], BF16, name="zt", tag="zt")
            zcol = pp.tile([P, 4], F32, name="zcol", tag="zcol")

            # memsets first: scratch gates the PE warmup, kvt/Zb hold zeros
            # that the boundary only partially overwrites.
            nc.vector.memset(scratch[:, :], 0.0)
            nc.vector.memset(ones_col[:, :], 1.0)
            for j in range(4):
                nc.vector.memset(kvt[j][:, :], 0.0)
                nc.vector.memset(Zb[j][:, :], 0.0)

            # ---- DMAs, ordered by first use (each splits over 16 queues) ----
            nc.sync.dma_start(out=w1qsb[:, 0:1, :], in_=w1q[:, 0:DIM])
            nc.sync.dma_start(out=xsb[0][:, 0:4, :], in_=xP[:, 0:2048])
            nc.sync.dma_start(out=xsb[0][:, 4:8, :], in_=xP[:, 2048:4096])
            nc.sync.dma_start(out=w1qsb[:, 1:4, :], in_=w1q[:, DIM:4 * DIM])
            nc.sync.dma_start(out=w1ksb[:, :, :], in_=w1k[:, :])
            nc.sync.dma_start(
                out=xsb[1][:, :, :], in_=xP[:, 4096:2 * 4096]
            )
            nc.sync.dma_start(out=w1vsb[:, :, :], in_=w1v[:, :])
            nc.sync.dma_start(
                out=xsb[2][:, :, :], in_=xP[:, 2 * 4096:3 * 4096]
            )
            nc.sync.dma_start(out=w2sb[:, :, :], in_=w2[:, :])
            nc.sync.dma_start(out=ec_sb[:, :], in_=ec[:, :])
            for ib in range(3, NTB):
                nc.sync.dma_start(
                    out=xsb[ib][:, :, :],
                    in_=xP[:, ib * 4096:(ib + 1) * 4096],
                )

            phase_a(nc, tc, NTB, xsb, w1qsb, w1ksb, w1vsb, scratch,
                    qt, kvt, zt, ones_col)
            phase_b(nc, tc, NTB, w2sb, ec_sb, ones_col, y,
                    qt, kvt, zt, zcol, Zb, Ms)

    nc.compile()
    return nc


def phase_a(nc, tc, NTB, xsb, w1qsb, w1ksb, w1vsb, scratch,
            qt, kvt, zt, ones_col):
    with (
        tc.tile_pool(name="phA_sb", bufs=3) as pa,
        tc.tile_pool(name="proj_ps", bufs=6, space="PSUM") as proj_ps,
        tc.tile_pool(name="hold_ps", bufs=1, space="PSUM") as hold_ps,
    ):
        # PSUM accumulators held across all of phase A (one bank each).
        # kvps holds 4 interleaved accumulation regions; zero it up
        # front and accumulate with start=False everywhere (hardware
        # bank-clear on start would wipe sibling regions).
        kvps = hold_ps.tile([P, 4 * P], F32, name="kvps", tag="kvps")
        zps = hold_ps.tile([1, CH], F32, name="zps", tag="zps")
        nc.vector.memset(kvps[:, :], 0.0)

        # PE warmup: dummy matmuls on zeroed scratch keep the PE busy from
        # ~t=0.5us so the HAM clock-gate is at 8/8 when real data lands,
        # and fill the otherwise-idle DMA ramp.
        for w in range(WARMUP_MMS):
            wps = proj_ps.tile([P, 512], F32, name=f"warm_{w}", tag="proj")
            nc.tensor.matmul(
                wps[:, :], scratch[:, 0:P], scratch[:, :],
                start=True, stop=True,
            )

        def elu1(out_ap, src_ps, tag_sfx, nm):
            """out = elu(src)+1 = max(src + 1, exp(min(src, 0))), bf16 out."""
            m_ = pa.tile([P, 512], F32, name=f"m_{nm}", tag=f"m{tag_sfx}")
            e_ = pa.tile([P, 512], F32, name=f"e_{nm}", tag=f"e{tag_sfx}")
            nc.vector.tensor_scalar_min(m_[:, :], src_ps[:, :], 0.0)
            nc.scalar.activation(e_[:, :], m_[:, :], AF.Exp)
            nc.vector.scalar_tensor_tensor(
                out_ap, src_ps[:, :], 1.0, e_[:, :], ALU.add, ALU.max
            )

        def zkv_block(ib, ksb, vsb):
            """z/kv accumulation for block ib; deferred one block so the
            k/v eviction chains never stall the PE."""
            first = (ib == 0)
            last = (ib == NTB - 1)
            for t in range(4):
                csl_t = slice(t * P, (t + 1) * P)
                # z += ones^T k   [1, 512]
                nc.tensor.matmul(
                    zps[0:1, :], ones_col[:, :], ksb[t][:, :],
                    start=(first and t == 0), stop=(last and t == 3),
                    skip_group_check=True,
                )
                # kvT[j] += v_pair^T k_pair   [128, 128] per head-pair.
                # One accumulation group for the whole packed bank:
                # start=False everywhere, per-element has_written handles
                # first-write-overwrite onto the memset zeros.
                for j in range(4):
                    csl = slice(j * P, (j + 1) * P)
                    nc.tensor.matmul(
                        kvps[:, csl], vsb[t][:, csl], ksb[t][:, csl],
                        start=False, stop=(last and t == 3 and j == 3),
                        skip_group_check=True,
                    )

        # ---- phase A: projections + kv/z accumulation ----
        prev_kv = None
        for ib in range(NTB):
            tsl = slice(ib * 512, (ib + 1) * 512)

            # q projection (c-major) with elu+1, into persistent qt
            for j in range(4):
                qps = proj_ps.tile([P, 512], F32, name=f"qps_{ib}_{j}", tag="proj")
                for ct in range(8):
                    nc.tensor.matmul(
                        qps[:, :],
                        w1qsb[:, j:j + 1, ct * P:(ct + 1) * P],
                        xsb[ib][:, ct:ct + 1, :],
                        start=(ct == 0),
                        stop=(ct == 7),
                    )
                elu1(qt[j][:, tsl], qps, "q", f"q_{ib}_{j}")

            # z/kv for the PREVIOUS block: its k/v evictions finished
            # while this block's q matmuls were streaming.
            if prev_kv is not None:
                zkv_block(ib - 1, *prev_kv)

            # k, v projections (token-major) per 128-token block
            ksb, vsb = [], []
            for t in range(4):
                tok = slice(t * P, (t + 1) * P)
                kps = proj_ps.tile([P, 512], F32, name=f"kps_{ib}_{t}", tag="proj")
                for ct in range(8):
                    nc.tensor.matmul(
                        kps[:, :],
                        xsb[ib][:, ct:ct + 1, tok],
                        w1ksb[:, ct:ct + 1, :],
                        start=(ct == 0),
                        stop=(ct == 7),
                    )
                k_sb = pa.tile(
                    [P, 512], BF16, name=f"k_{ib}_{t}", tag="k_sb", bufs=9
                )
                elu1(k_sb[:, :], kps, "k", f"k_{ib}_{t}")
                ksb.append(k_sb)

                vps = proj_ps.tile([P, 512], F32, name=f"vps_{ib}_{t}", tag="proj")
                for ct in range(8):
                    nc.tensor.matmul(
                        vps[:, :],
                        xsb[ib][:, ct:ct + 1, tok],
                        w1vsb[:, ct:ct + 1, :],
                        start=(ct == 0),
                        stop=(ct == 7),
                    )
                v_sb = pa.tile(
                    [P, 512], BF16, name=f"v_{ib}_{t}", tag="v_sb", bufs=9
                )
                nc.scalar.copy(v_sb[:, :], vps[:, :])
                vsb.append(v_sb)
            prev_kv = (ksb, vsb)

        zkv_block(NTB - 1, *prev_kv)

        # ---- evict PSUM accumulators before releasing phase-A pools ----
        # kvt holds memset zeros; only the diagonal 64-blocks are written.
        # Alternate DVE/ACT so the boundary doesn't serialize on one queue.
        nc.vector.tensor_copy(zt[0:1, :], zps[0:1, :])
        for j in range(4):
            eng = nc.vector.tensor_copy if j % 2 == 0 else nc.scalar.copy
            eng(kvt[j][0:64, 0:64], kvps[0:64, j * P:j * P + 64])
            eng(
                kvt[j][64:128, 64:128],
                kvps[64:128, j * P + 64:(j + 1) * P],
            )


def phase_b(nc, tc, NTB, w2sb, ec_sb, ones_col, y,
            qt, kvt, zt, zcol, Zb, Ms):
    with (
        tc.tile_pool(name="phB_sb", bufs=2) as pb,
        tc.tile_pool(name="qsc_pool", bufs=8) as qp,
        tc.tile_pool(name="d_ps", bufs=2, space="PSUM") as d_ps,
        tc.tile_pool(name="bc_ps", bufs=2, space="PSUM") as bc_ps,
        tc.tile_pool(name="y_ps", bufs=4, space="PSUM") as y_ps,
    ):
        # ---- Zb first: z row -> per-partition columns via K=1 matmuls ----
        # zc[:, j] = zt[0, j*128:(j+1)*128]^T  (plain matmul with N=1)
        zc = bc_ps.tile([P, 512], F32, name="zc", tag="bc")
        nc.vector.memset(zc[:, 0:4], 0.0)
        for j in range(4):
            nc.tensor.matmul(
                zc[:, j:j + 1],
                zt[0:1, j * P:(j + 1) * P],
                ones_col[0:1, 0:1],
                start=False, stop=(j == 3), skip_group_check=True,
            )
        nc.vector.tensor_copy(zcol[:, :], zc[:, 0:4])
        for j in range(4):
            nc.vector.tensor_copy(
                Zb[j][0:64, 2 * j:2 * j + 1], zcol[0:64, j:j + 1]
            )
            nc.vector.tensor_copy(
                Zb[j][64:128, 2 * j + 1:2 * j + 2], zcol[64:128, j:j + 1]
            )

        def den_recip(ib):
            """den = Zb^T q -> r = 1/den (fast approx), bf16 rT."""
            tsl = slice(ib * 512, (ib + 1) * 512)
            dps = d_ps.tile([8, 512], F32, name=f"dps_{ib}", tag="d")
            for j in range(4):
                nc.tensor.matmul(
                    dps[:, :], Zb[j][:, :], qt[j][:, tsl],
                    start=(j == 0), stop=(j == 3),
                )
            rf = pb.tile([8, 512], F32, name=f"rf_{ib}", tag="rf")
            nc.vector.reciprocal_approx_fast(out=rf[:, :], in_=dps[:, :])
            rT = pb.tile([8, 512], BF16, name=f"rT_{ib}", tag="rT")
            nc.scalar.copy(rT[:, :], rf[:, :])
            return rT

        def bc_qsc(ib, rT):
            """rbc = E^T r broadcast; qsc = qt * rbc (bf16)."""
            tsl = slice(ib * 512, (ib + 1) * 512)
            qsc = []
            for j in range(4):
                bcp = bc_ps.tile([P, 512], F32, name=f"bcp_{ib}_{j}", tag="bc")
                nc.tensor.matmul(
                    bcp[:, :], ec_sb[:, j * P:(j + 1) * P], rT[:, :],
                    start=True, stop=True,
                )
                qs = qp.tile([P, 512], BF16, name=f"qsc_{ib}_{j}", tag="qsc")
                nc.vector.tensor_mul(qs[:, :], qt[j][:, tsl], bcp[:, :])
                qsc.append(qs)
            return qsc

        # den(0) immediately after Zb, then M matmuls (kvt-gated) cover
        # the recip/rT latency; den(1)/bc(0) complete the 2-deep prologue.
        rTs = {0: den_recip(0)}

        # M = kvT^T @ W2 per ch-tile (PSUM tiles share the y tag budget)
        for j in range(4):
            for h in range(2):
                hsl = slice(h * 512, (h + 1) * 512)
                mps = y_ps.tile([P, 512], F32, name=f"mps_{j}_{h}", tag="y")
                nc.tensor.matmul(
                    mps[:, :], kvt[j][:, :], w2sb[:, j:j + 1, hsl],
                    start=True, stop=True,
                )
                if h == 0:
                    nc.vector.tensor_copy(Ms[j][:, hsl], mps[:, :])
                else:
                    nc.scalar.copy(Ms[j][:, hsl], mps[:, :])

        rTs[1] = den_recip(1)
        qscs = {0: bc_qsc(0, rTs.pop(0))}

        # ---- fused main loop: den(ib+2) / bc+qsc(ib+1) / y(ib) ----
        for ib in range(NTB):
            if ib + 2 < NTB:
                rTs[ib + 2] = den_recip(ib + 2)
            if ib + 1 < NTB:
                qscs[ib + 1] = bc_qsc(ib + 1, rTs.pop(ib + 1))

            qsc = qscs.pop(ib)
            for t in range(4):
                tok = slice(t * P, (t + 1) * P)
                row = (ib * 4 + t) * P
                yp = [
                    y_ps.tile([P, 512], F32, name=f"yps_{ib}_{t}_{h}", tag="y")
                    for h in range(2)
                ]
                for j in range(4):
                    for h in range(2):
                        hsl = slice(h * 512, (h + 1) * 512)
                        nc.tensor.matmul(
                            yp[h][:, :], qsc[j][:, tok], Ms[j][:, hsl],
                            start=(j == 0), stop=(j == 3),
                        )
                y_sb = pb.tile(
                    [P, DIM], F32, name=f"y_{ib}_{t}", tag="y_sb", bufs=3
                )
                nc.vector.tensor_copy(y_sb[:, 0:512], yp[0][:, :])
                nc.scalar.copy(y_sb[:, 512:1024], yp[1][:, :])
                nc.sync.dma_start(out=y[row:row + P, :], in_=y_sb[:, :])


_NC_CACHE = {}


def _get_nc(T=T_FULL):
    if T not in _NC_CACHE:
        _NC_CACHE[T] = build_nc(T)
    return _NC_CACHE[T]


def make_in_maps(x, W_qkv, W_out, b_out):
    import ml_dtypes

    bf16 = ml_dtypes.bfloat16
    x = np.asarray(x, dtype=np.float32)
    W_qkv = np.asarray(W_qkv, dtype=np.float32).astype(bf16)
    W_out = np.asarray(W_out, dtype=np.float32).astype(bf16)
    NTB = T_FULL // 512

    # xP[p, ib*4096 + ct*512 + tl] = x[b][ib*512 + tl, ct*128 + p]
    xPs = []
    for b in range(B):
        a = x[b].astype(bf16).reshape(NTB, 512, 8, P)
        xPs.append(
            np.ascontiguousarray(
                a.transpose(3, 0, 2, 1).reshape(P, NTB * 8 * 512)
            )
        )

    w1qs, w1ks, w1vs, w2s = [], [], [], []
    for hg in range(2):
        cs = slice(hg * CH, (hg + 1) * CH)
        Wq = W_qkv[:, cs]                                   # [1024, 512]
        Wk = W_qkv[:, DIM + hg * CH:DIM + (hg + 1) * CH]
        Wv = W_qkv[:, 2 * DIM + hg * CH:2 * DIM + (hg + 1) * CH]
        # w1q[p, j*1024 + ct*128 + c] = Wq[ct*128 + p, j*128 + c]
        w1qs.append(
            np.ascontiguousarray(
                Wq.reshape(8, P, 4, P).transpose(1, 2, 0, 3).reshape(P, 4 * DIM)
            )
        )
        # w1k[p, ct*512 + n] = Wk[ct*128 + p, n]
        w1ks.append(
            np.ascontiguousarray(
                Wk.reshape(8, P, CH).transpose(1, 0, 2).reshape(P, 8 * CH)
            )
        )
        w1vs.append(
            np.ascontiguousarray(
                Wv.reshape(8, P, CH).transpose(1, 0, 2).reshape(P, 8 * CH)
            )
        )
        # w2[p, j*1024 + n] = W_out[hg*512 + j*128 + p, n]
        w2s.append(
            np.ascontiguousarray(
                W_out[cs, :].reshape(4, P, DIM).transpose(1, 0, 2).reshape(P, 4 * DIM)
            )
        )
    ecm = make_ec().astype(bf16)

    in_maps = []
    for core in range(N_CORES):
        b, hg = core // 2, core % 2
        in_maps.append({
            "xP": xPs[b], "w1q": w1qs[hg], "w1k": w1ks[hg],
            "w1v": w1vs[hg], "w2": w2s[hg], "ec": ecm,
        })
    return in_maps


def make_ec():
    """E selector: ec[h, j*128+p] = 1 iff head-of-partition-p-in-tile-j == h."""
    ecm = np.zeros((8, CH), dtype=np.float32)
    for j in range(4):
        ecm[2 * j, j * P:j * P + 64] = 1.0
        ecm[2 * j + 1, j * P + 64:(j + 1) * P] = 1.0
    return ecm


def kernel(x, W_qkv, W_out, b_out):
    from concourse.bass_utils import run_bass_kernel_spmd

    nc = _get_nc(T_FULL)
    in_maps = make_in_maps(x, W_qkv, W_out, b_out)
    res = run_bass_kernel_spmd(nc, in_maps, core_ids=list(range(N_CORES))).results
    bo = np.asarray(b_out, dtype=np.float32)
    out = np.empty((B, T_FULL, DIM), dtype=np.float32)
    for b in range(B):
        out[b] = res[2 * b]["y"] + res[2 * b + 1]["y"] + bo
    return out


# revision 29
# speedup vs baseline: 1.0006x; 1.0006x over previous
"""Linear attention kernel for 8 Trainium2 NeuronCores.

Sharding: core = 2*b + hg  (b in 0..3 batches, hg in 0..1 head-groups of 8 heads).
Fully data-parallel - no collectives; host sums the two head-group partials per
batch. Each core adds bias/2 so the pair-sum carries the full bias.

Per-core math (T=4096 tokens, CH=512 = 8 heads x 64, DIM=1024):
  qT   = (x @ Wq)^T          c-major [CH, T], elu+1
  k,v  = x @ Wk, x @ Wv      token-major [T, CH], elu+1 on k
  kvT  = v^T k  (per head-pair, diagonal 64-blocks valid), accumulated in PSUM
  z    = ones^T k            [1, CH], accumulated in PSUM
  den  = Zblock^T qT         [8, T]   (Zblock = block-diag scatter of z)
  r    = 1/den;  rbc = E^T r (broadcast r over each head's 64 rows)
  qsc  = qT * rbc
  M    = kvT^T @ W2  (per 128-row ch-tile; off-diag blocks of kvT zeroed)
  y    = qsc^T @ M + bias/2  token-major [T, DIM]

v2 scheduling: batched 3D-AP DMAs ordered by first use, PE warmup matmuls
during the DMA ramp (HAM stays warm), 3-op elu+1 (min/exp/stt-max), z
transposed to columns via K=1 matmuls at the phase boundary (no SB->SB DMA),
and a fused phase B with 2-stage lookahead so den/bc/recip of ib+1/ib+2
overlap the y matmuls of ib and the y writeback DMA is spread evenly.
"""

import sys

sys.path.insert(0, "/opt/trn_rl_repo")

import numpy as np

import concourse.bass as bass
import concourse.mybir as mybir
import concourse.tile as tile
from concourse import bacc

F32 = mybir.dt.float32
BF16 = mybir.dt.bfloat16
AF = mybir.ActivationFunctionType
ALU = mybir.AluOpType

DIM = 1024      # model dim (contraction for projections)
CH = 512        # per-core channels (8 heads x 64)
P = 128

N_CORES = 8
B, T_FULL = 4, 4096

WARMUP_MMS = 12


def build_nc(T=T_FULL):
    NTB = T // 512          # 512-token blocks
    nc = bacc.Bacc(None, target_bir_lowering=False, debug=False)

    # All inputs host-repacked partition-major so every DMA line is a
    # contiguous 2-8KB row segment (packet-count, not bandwidth, limits
    # the startup ramp).
    xP = nc.declare_dram_parameter("xP", [P, NTB * 8 * 512], BF16, isOutput=False)
    w1q = nc.declare_dram_parameter("w1q", [P, 4 * 1024], BF16, isOutput=False)
    w1k = nc.declare_dram_parameter("w1k", [P, 8 * 512], BF16, isOutput=False)
    w1v = nc.declare_dram_parameter("w1v", [P, 8 * 512], BF16, isOutput=False)
    w2 = nc.declare_dram_parameter("w2", [P, 4 * 1024], BF16, isOutput=False)
    ec = nc.declare_dram_parameter("ec", [8, CH], BF16, isOutput=False)
    y = nc.declare_dram_parameter("y", [T, DIM], F32, isOutput=True)

    with tile.TileContext(nc) as tc:
        with tc.tile_pool(name="persist", bufs=1) as pp:
            # ---- persistent tiles ----
            scratch = pp.tile([P, 512], BF16, name="scratch", tag="scratch")
            ones_col = pp.tile([P, 1], BF16, name="ones_col", tag="ones_col")
            w1qsb = pp.tile([P, 4, DIM], BF16, name="w1qsb", tag="w1qsb")
            w1ksb = pp.tile([P, 8, CH], BF16, name="w1ksb", tag="w1ksb")
            w1vsb = pp.tile([P, 8, CH], BF16, name="w1vsb", tag="w1vsb")
            w2sb = pp.tile([P, 4, DIM], BF16, name="w2sb", tag="w2sb")
            ec_sb = pp.tile([8, CH], BF16, name="ec_sb", tag="ec_sb")
            xsb = [
                pp.tile([P, 8, 512], BF16, name=f"xsb_{ib}", tag=f"xsb_{ib}")
                for ib in range(NTB)
            ]
            qt = [
                pp.tile([P, T], BF16, name=f"qt_{j}", tag=f"qt_{j}")
                for j in range(4)
            ]
            kvt = [
                pp.tile([P, P], BF16, name=f"kvt_{j}", tag=f"kvt_{j}")
                for j in range(4)
            ]
            Zb = [
                pp.tile([P, 8], BF16, name=f"Zb_{j}", tag=f"Zb_{j}")
                for j in range(4)
            ]
            Ms = [
                pp.tile([P, DIM], BF16, name=f"Ms_{j}", tag=f"Ms_{j}")
                for j in range(4)
            ]
            zt = pp.tile([1, CH], BF16, name="zt", tag="zt")
            zcol = pp.tile([P, 4], F32, name="zcol", tag="zcol")

            # memsets first: scratch gates the PE warmup, kvt/Zb hold zeros
            # that the boundary only partially overwrites.
            nc.vector.memset(scratch[:, :], 0.0)
            nc.vector.memset(ones_col[:, :], 1.0)
            for j in range(4):
                nc.vector.memset(kvt[j][:, :], 0.0)
                nc.vector.memset(Zb[j][:, :], 0.0)

            # ---- DMAs, ordered by first use (each splits over 16 queues) ----
            nc.sync.dma_start(out=w1qsb[:, 0:1, :], in_=w1q[:, 0:DIM])
            nc.sync.dma_start(out=xsb[0][:, 0:4, :], in_=xP[:, 0:2048])
            nc.sync.dma_start(out=xsb[0][:, 4:8, :], in_=xP[:, 2048:4096])
            nc.sync.dma_start(out=w1qsb[:, 1:4, :], in_=w1q[:, DIM:4 * DIM])
            nc.sync.dma_start(out=w1ksb[:, :, :], in_=w1k[:, :])
            nc.sync.dma_start(
                out=xsb[1][:, :, :], in_=xP[:, 4096:2 * 4096]
            )
            nc.sync.dma_start(out=w1vsb[:, :, :], in_=w1v[:, :])
            nc.sync.dma_start(
                out=xsb[2][:, :, :], in_=xP[:, 2 * 4096:3 * 4096]
            )
            nc.sync.dma_start(out=w2sb[:, :, :], in_=w2[:, :])
            nc.sync.dma_start(out=ec_sb[:, :], in_=ec[:, :])
            for ib in range(3, NTB):
                nc.sync.dma_start(
                    out=xsb[ib][:, :, :],
                    in_=xP[:, ib * 4096:(ib + 1) * 4096],
                )

            phase_a(nc, tc, NTB, xsb, w1qsb, w1ksb, w1vsb, scratch,
                    qt, kvt, zt, ones_col)
            phase_b(nc, tc, NTB, w2sb, ec_sb, ones_col, y,
                    qt, kvt, zt, zcol, Zb, Ms)

    nc.compile()
    return nc


def phase_a(nc, tc, NTB, xsb, w1qsb, w1ksb, w1vsb, scratch,
            qt, kvt, zt, ones_col):
    with (
        tc.tile_pool(name="phA_sb", bufs=3) as pa,
        tc.tile_pool(name="proj_ps", bufs=6, space="PSUM") as proj_ps,
        tc.tile_pool(name="hold_ps", bufs=1, space="PSUM") as hold_ps,
    ):
        # PSUM accumulators held across all of phase A (one bank each).
        # kvps holds 4 interleaved accumulation regions; zero it up
        # front and accumulate with start=False everywhere (hardware
        # bank-clear on start would wipe sibling regions).
        kvps = hold_ps.tile([P, 4 * P], F32, name="kvps", tag="kvps")
        zps = hold_ps.tile([1, CH], F32, name="zps", tag="zps")
        nc.vector.memset(kvps[:, :], 0.0)

        # PE warmup: dummy matmuls on zeroed scratch keep the PE busy from
        # ~t=0.5us so the HAM clock-gate is at 8/8 when real data lands,
        # and fill the otherwise-idle DMA ramp.
        for w in range(WARMUP_MMS):
            wps = proj_ps.tile([P, 512], F32, name=f"warm_{w}", tag="proj")
            nc.tensor.matmul(
                wps[:, :], scratch[:, 0:P], scratch[:, :],
                start=True, stop=True,
            )

        def elu1(out_ap, src_ps, tag_sfx, nm):
            """out = elu(src)+1 = max(src + 1, exp(min(src, 0))), bf16 out."""
            m_ = pa.tile([P, 512], F32, name=f"m_{nm}", tag=f"m{tag_sfx}")
            e_ = pa.tile([P, 512], F32, name=f"e_{nm}", tag=f"e{tag_sfx}")
            nc.vector.tensor_scalar_min(m_[:, :], src_ps[:, :], 0.0)
            nc.scalar.activation(e_[:, :], m_[:, :], AF.Exp)
            nc.vector.scalar_tensor_tensor(
                out_ap, src_ps[:, :], 1.0, e_[:, :], ALU.add, ALU.max
            )

        def zkv_block(ib, ksb, vsb):
            """z/kv accumulation for block ib; deferred one block so the
            k/v eviction chains never stall the PE."""
            first = (ib == 0)
            last = (ib == NTB - 1)
            for t in range(4):
                csl_t = slice(t * P, (t + 1) * P)
                # z += ones^T k   [1, 512]
                nc.tensor.matmul(
                    zps[0:1, :], ones_col[:, :], ksb[t][:, :],
                    start=(first and t == 0), stop=(last and t == 3),
                    skip_group_check=True,
                )
                # kvT[j] += v_pair^T k_pair   [128, 128] per head-pair.
                # One accumulation group for the whole packed bank:
                # start=False everywhere, per-element has_written handles
                # first-write-overwrite onto the memset zeros.
                for j in range(4):
                    csl = slice(j * P, (j + 1) * P)
                    nc.tensor.matmul(
                        kvps[:, csl], vsb[t][:, csl], ksb[t][:, csl],
                        start=False, stop=(last and t == 3 and j == 3),
                        skip_group_check=True,
                    )

        # ---- phase A: projections + kv/z accumulation ----
        prev_kv = None
        for ib in range(NTB):
            tsl = slice(ib * 512, (ib + 1) * 512)

            # q projection (c-major) with elu+1, into persistent qt
            for j in range(4):
                qps = proj_ps.tile([P, 512], F32, name=f"qps_{ib}_{j}", tag="proj")
                for ct in range(8):
                    nc.tensor.matmul(
                        qps[:, :],
                        w1qsb[:, j:j + 1, ct * P:(ct + 1) * P],
                        xsb[ib][:, ct:ct + 1, :],
                        start=(ct == 0),
                        stop=(ct == 7),
                    )
                elu1(qt[j][:, tsl], qps, "q", f"q_{ib}_{j}")

            # z/kv for the PREVIOUS block: its k/v evictions finished
            # while this block's q matmuls were streaming.
            if prev_kv is not None:
                zkv_block(ib - 1, *prev_kv)

            # k, v projections (token-major) per 128-token block
            ksb, vsb = [], []
            for t in range(4):
                tok = slice(t * P, (t + 1) * P)
                kps = proj_ps.tile([P, 512], F32, name=f"kps_{ib}_{t}", tag="proj")
                for ct in range(8):
                    nc.tensor.matmul(
                        kps[:, :],
                        xsb[ib][:, ct:ct + 1, tok],
                        w1ksb[:, ct:ct + 1, :],
                        start=(ct == 0),
                        stop=(ct == 7),
                    )
                k_sb = pa.tile(
                    [P, 512], BF16, name=f"k_{ib}_{t}", tag="k_sb", bufs=9
                )
                elu1(k_sb[:, :], kps, "k", f"k_{ib}_{t}")
                ksb.append(k_sb)

                vps = proj_ps.tile([P, 512], F32, name=f"vps_{ib}_{t}", tag="proj")
                for ct in range(8):
                    nc.tensor.matmul(
                        vps[:, :],
                        xsb[ib][:, ct:ct + 1, tok],
                        w1vsb[:, ct:ct + 1, :],
                        start=(ct == 0),
                        stop=(ct == 7),
                    )
                v_sb = pa.tile(
                    [P, 512], BF16, name=f"v_{ib}_{t}", tag="v_sb", bufs=9
                )
                nc.scalar.copy(v_sb[:, :], vps[:, :])
                vsb.append(v_sb)
            prev_kv = (ksb, vsb)

        zkv_block(NTB - 1, *prev_kv)

        # ---- evict PSUM accumulators before releasing phase-A pools ----
        # kvt holds memset zeros; only the diagonal 64-blocks are written.
        # Alternate DVE/ACT so the boundary doesn't serialize on one queue.
        nc.vector.tensor_copy(zt[0:1, :], zps[0:1, :])
        for j in range(4):
            eng = nc.vector.tensor_copy if j % 2 == 0 else nc.scalar.copy
            eng(kvt[j][0:64, 0:64], kvps[0:64, j * P:j * P + 64])
            eng(
                kvt[j][64:128, 64:128],
                kvps[64:128, j * P + 64:(j + 1) * P],
            )


def phase_b(nc, tc, NTB, w2sb, ec_sb, ones_col, y,
            qt, kvt, zt, zcol, Zb, Ms):
    with (
        tc.tile_pool(name="phB_sb", bufs=2) as pb,
        tc.tile_pool(name="qsc_pool", bufs=8) as qp,
        tc.tile_pool(name="d_ps", bufs=2, space="PSUM") as d_ps,
        tc.tile_pool(name="bc_ps", bufs=2, space="PSUM") as bc_ps,
        tc.tile_pool(name="y_ps", bufs=4, space="PSUM") as y_ps,
    ):
        # ---- Zb first: z row -> per-partition columns via K=1 matmuls ----
        # zc[:, j] = zt[0, j*128:(j+1)*128]^T  (plain matmul with N=1)
        zc = bc_ps.tile([P, 512], F32, name="zc", tag="bc")
        nc.vector.memset(zc[:, 0:4], 0.0)
        for j in range(4):
            nc.tensor.matmul(
                zc[:, j:j + 1],
                zt[0:1, j * P:(j + 1) * P],
                ones_col[0:1, 0:1],
                start=False, stop=(j == 3), skip_group_check=True,
            )
        nc.vector.tensor_copy(zcol[:, :], zc[:, 0:4])
        for j in range(4):
            nc.vector.tensor_copy(
                Zb[j][0:64, 2 * j:2 * j + 1], zcol[0:64, j:j + 1]
            )
            nc.vector.tensor_copy(
                Zb[j][64:128, 2 * j + 1:2 * j + 2], zcol[64:128, j:j + 1]
            )

        def den_recip(ib):
            """den = Zb^T q -> r = 1/den (fast approx), bf16 rT."""
            tsl = slice(ib * 512, (ib + 1) * 512)
            dps = d_ps.tile([8, 512], F32, name=f"dps_{ib}", tag="d")
            for j in range(4):
                nc.tensor.matmul(
                    dps[:, :], Zb[j][:, :], qt[j][:, tsl],
                    start=(j == 0), stop=(j == 3),
                )
            rf = pb.tile([8, 512], F32, name=f"rf_{ib}", tag="rf")
            nc.vector.reciprocal_approx_fast(out=rf[:, :], in_=dps[:, :])
            rT = pb.tile([8, 512], BF16, name=f"rT_{ib}", tag="rT")
            nc.scalar.copy(rT[:, :], rf[:, :])
            return rT

        def bc_qsc(ib, rT):
            """rbc = E^T r broadcast; qsc = qt * rbc (bf16)."""
            tsl = slice(ib * 512, (ib + 1) * 512)
            qsc = []
            for j in range(4):
                bcp = bc_ps.tile([P, 512], F32, name=f"bcp_{ib}_{j}", tag="bc")
                nc.tensor.matmul(
                    bcp[:, :], ec_sb[:, j * P:(j + 1) * P], rT[:, :],
                    start=True, stop=True,
                )
                qs = qp.tile([P, 512], BF16, name=f"qsc_{ib}_{j}", tag="qsc")
                nc.vector.tensor_mul(qs[:, :], qt[j][:, tsl], bcp[:, :])
                qsc.append(qs)
            return qsc

        # M = kvT^T @ W2 per ch-tile: kvt-gated, runs while the Zb DVE
        # chain completes; then den/recip prologue (2 deep).
        for j in range(4):
            for h in range(2):
                hsl = slice(h * 512, (h + 1) * 512)
                mps = y_ps.tile([P, 512], F32, name=f"mps_{j}_{h}", tag="y")
                nc.tensor.matmul(
                    mps[:, :], kvt[j][:, :], w2sb[:, j:j + 1, hsl],
                    start=True, stop=True,
                )
                if h == 0:
                    nc.vector.tensor_copy(Ms[j][:, hsl], mps[:, :])
                else:
                    nc.scalar.copy(Ms[j][:, hsl], mps[:, :])

        rTs = {0: den_recip(0)}
        rTs[1] = den_recip(1)
        qscs = {0: bc_qsc(0, rTs.pop(0))}

        # ---- fused main loop: den(ib+2) / bc+qsc(ib+1) / y(ib) ----
        for ib in range(NTB):
            if ib + 2 < NTB:
                rTs[ib + 2] = den_recip(ib + 2)
            if ib + 1 < NTB:
                qscs[ib + 1] = bc_qsc(ib + 1, rTs.pop(ib + 1))

            qsc = qscs.pop(ib)
            for t in range(4):
                tok = slice(t * P, (t + 1) * P)
                row = (ib * 4 + t) * P
                yp = [
                    y_ps.tile([P, 512], F32, name=f"yps_{ib}_{t}_{h}", tag="y")
                    for h in range(2)
                ]
                for j in range(4):
                    for h in range(2):
                        hsl = slice(h * 512, (h + 1) * 512)
                        nc.tensor.matmul(
                            yp[h][:, :], qsc[j][:, tok], Ms[j][:, hsl],
                            start=(j == 0), stop=(j == 3),
                        )
                y_sb = pb.tile(
                    [P, DIM], F32, name=f"y_{ib}_{t}", tag="y_sb", bufs=3
                )
                nc.vector.tensor_copy(y_sb[:, 0:512], yp[0][:, :])
                nc.scalar.copy(y_sb[:, 512:1024], yp[1][:, :])
                nc.sync.dma_start(out=y[row:row + P, :], in_=y_sb[:, :])


_NC_CACHE = {}


def _get_nc(T=T_FULL):
    if T not in _NC_CACHE:
        _NC_CACHE[T] = build_nc(T)
    return _NC_CACHE[T]


def make_in_maps(x, W_qkv, W_out, b_out):
    import ml_dtypes

    bf16 = ml_dtypes.bfloat16
    x = np.asarray(x, dtype=np.float32)
    W_qkv = np.asarray(W_qkv, dtype=np.float32).astype(bf16)
    W_out = np.asarray(W_out, dtype=np.float32).astype(bf16)
    NTB = T_FULL // 512

    # xP[p, ib*4096 + ct*512 + tl] = x[b][ib*512 + tl, ct*128 + p]
    xPs = []
    for b in range(B):
        a = x[b].astype(bf16).reshape(NTB, 512, 8, P)
        xPs.append(
            np.ascontiguousarray(
                a.transpose(3, 0, 2, 1).reshape(P, NTB * 8 * 512)
            )
        )

    w1qs, w1ks, w1vs, w2s = [], [], [], []
    for hg in range(2):
        cs = slice(hg * CH, (hg + 1) * CH)
        Wq = W_qkv[:, cs]                                   # [1024, 512]
        Wk = W_qkv[:, DIM + hg * CH:DIM + (hg + 1) * CH]
        Wv = W_qkv[:, 2 * DIM + hg * CH:2 * DIM + (hg + 1) * CH]
        # w1q[p, j*1024 + ct*128 + c] = Wq[ct*128 + p, j*128 + c]
        w1qs.append(
            np.ascontiguousarray(
                Wq.reshape(8, P, 4, P).transpose(1, 2, 0, 3).reshape(P, 4 * DIM)
            )
        )
        # w1k[p, ct*512 + n] = Wk[ct*128 + p, n]
        w1ks.append(
            np.ascontiguousarray(
                Wk.reshape(8, P, CH).transpose(1, 0, 2).reshape(P, 8 * CH)
            )
        )
        w1vs.append(
            np.ascontiguousarray(
                Wv.reshape(8, P, CH).transpose(1, 0, 2).reshape(P, 8 * CH)
            )
        )
        # w2[p, j*1024 + n] = W_out[hg*512 + j*128 + p, n]
        w2s.append(
            np.ascontiguousarray(
                W_out[cs, :].reshape(4, P, DIM).transpose(1, 0, 2).reshape(P, 4 * DIM)
            )
        )
    ecm = make_ec().astype(bf16)

    in_maps = []
    for core in range(N_CORES):
        b, hg = core // 2, core % 2
        in_maps.append({
            "xP": xPs[b], "w1q": w1qs[hg], "w1k": w1ks[hg],
            "w1v": w1vs[hg], "w2": w2s[hg], "ec": ecm,
        })
    return in_maps


def make_ec():
    """E selector: ec[h, j*128+p] = 1 iff head-of-partition-p-in-tile-j == h."""
    ecm = np.zeros((8, CH), dtype=np.float32)
    for j in range(4):
        ecm[2 * j, j * P:j * P + 64] = 1.0
        ecm[2 * j + 1, j * P + 64:(j + 1) * P] = 1.0
    return ecm


def kernel(x, W_qkv, W_out, b_out):
    from concourse.bass_utils import run_bass_kernel_spmd

    nc = _get_nc(T_FULL)
    in_maps = make_in_maps(x, W_qkv, W_out, b_out)
    res = run_bass_kernel_spmd(nc, in_maps, core_ids=list(range(N_CORES))).results
    bo = np.asarray(b_out, dtype=np.float32)
    out = np.empty((B, T_FULL, DIM), dtype=np.float32)
    for b in range(B):
        out[b] = res[2 * b]["y"] + res[2 * b + 1]["y"] + bo
    return out


# revision 30
# speedup vs baseline: 1.1867x; 1.1860x over previous
"""Linear attention kernel for 8 Trainium2 NeuronCores.

Sharding: core = 2*b + hg  (b in 0..3 batches, hg in 0..1 head-groups of 8 heads).
Fully data-parallel - no collectives; host sums the two head-group partials per
batch. Each core adds bias/2 so the pair-sum carries the full bias.

Per-core math (T=4096 tokens, CH=512 = 8 heads x 64, DIM=1024):
  qT   = (x @ Wq)^T          c-major [CH, T], elu+1
  k,v  = x @ Wk, x @ Wv      token-major [T, CH], elu+1 on k
  kvT  = v^T k  (per head-pair, diagonal 64-blocks valid), accumulated in PSUM
  z    = ones^T k            [1, CH], accumulated in PSUM
  den  = Zblock^T qT         [8, T]   (Zblock = block-diag scatter of z)
  r    = 1/den;  rbc = E^T r (broadcast r over each head's 64 rows)
  qsc  = qT * rbc
  M    = kvT^T @ W2  (per 128-row ch-tile; off-diag blocks of kvT zeroed)
  y    = qsc^T @ M + bias/2  token-major [T, DIM]

v2 scheduling: batched 3D-AP DMAs ordered by first use, PE warmup matmuls
during the DMA ramp (HAM stays warm), 3-op elu+1 (min/exp/stt-max), z
transposed to columns via K=1 matmuls at the phase boundary (no SB->SB DMA),
and a fused phase B with 2-stage lookahead so den/bc/recip of ib+1/ib+2
overlap the y matmuls of ib and the y writeback DMA is spread evenly.
"""

import sys

sys.path.insert(0, "/opt/trn_rl_repo")

import numpy as np

import concourse.bass as bass
import concourse.mybir as mybir
import concourse.tile as tile
from concourse import bacc

F32 = mybir.dt.float32
BF16 = mybir.dt.bfloat16
AF = mybir.ActivationFunctionType
ALU = mybir.AluOpType

DIM = 1024      # model dim (contraction for projections)
CH = 512        # per-core channels (8 heads x 64)
P = 128

N_CORES = 8
B, T_FULL = 4, 4096

WARMUP_MMS = 12


def build_nc(T=T_FULL):
    NTB = T // 512          # 512-token blocks
    nc = bacc.Bacc(None, target_bir_lowering=False, debug=False)

    # All inputs host-repacked partition-major so every DMA line is a
    # contiguous 2-8KB row segment (packet-count, not bandwidth, limits
    # the startup ramp).
    xP = nc.declare_dram_parameter("xP", [P, NTB * 8 * 512], BF16, isOutput=False)
    w1q = nc.declare_dram_parameter("w1q", [P, 4 * 1024], BF16, isOutput=False)
    w1k = nc.declare_dram_parameter("w1k", [P, 8 * 512], BF16, isOutput=False)
    w1v = nc.declare_dram_parameter("w1v", [P, 8 * 512], BF16, isOutput=False)
    w2 = nc.declare_dram_parameter("w2", [P, 4 * 1024], BF16, isOutput=False)
    ec = nc.declare_dram_parameter("ec", [8, CH], BF16, isOutput=False)
    y = nc.declare_dram_parameter("y", [T, DIM], F32, isOutput=True)

    with tile.TileContext(nc) as tc:
        with tc.tile_pool(name="persist", bufs=1) as pp:
            # ---- persistent tiles ----
            scratch = pp.tile([P, 512], BF16, name="scratch", tag="scratch")
            ones_col = pp.tile([P, 1], BF16, name="ones_col", tag="ones_col")
            w1qsb = pp.tile([P, 4, DIM], BF16, name="w1qsb", tag="w1qsb")
            w1ksb = pp.tile([P, 8, CH], BF16, name="w1ksb", tag="w1ksb")
            w1vsb = pp.tile([P, 8, CH], BF16, name="w1vsb", tag="w1vsb")
            w2sb = pp.tile([P, 4, DIM], BF16, name="w2sb", tag="w2sb")
            ec_sb = pp.tile([8, CH], BF16, name="ec_sb", tag="ec_sb")
            xsb = [
                pp.tile([P, 8, 512], BF16, name=f"xsb_{ib}", tag=f"xsb_{ib}")
                for ib in range(NTB)
            ]
            qt = [
                pp.tile([P, T], BF16, name=f"qt_{j}", tag=f"qt_{j}")
                for j in range(4)
            ]
            kvt = [
                pp.tile([P, P], BF16, name=f"kvt_{j}", tag=f"kvt_{j}")
                for j in range(4)
            ]
            Zb = [
                pp.tile([P, 8], BF16, name=f"Zb_{j}", tag=f"Zb_{j}")
                for j in range(4)
            ]
            Ms = [
                pp.tile([P, DIM], BF16, name=f"Ms_{j}", tag=f"Ms_{j}")
                for j in range(4)
            ]
            zt = pp.tile([1, CH], BF16, name="zt", tag="zt")
            zcol = pp.tile([P, 4], F32, name="zcol", tag="zcol")

            # memsets first: scratch gates the PE warmup, kvt/Zb hold zeros
            # that the boundary only partially overwrites.
            nc.vector.memset(scratch[:, :], 0.0)
            nc.vector.memset(ones_col[:, :], 1.0)
            for j in range(4):
                nc.vector.memset(kvt[j][:, :], 0.0)
                nc.vector.memset(Zb[j][:, :], 0.0)

            # ---- DMAs, ordered by first use (each splits over 16 queues) ----
            nc.sync.dma_start(out=w1qsb[:, 0:1, :], in_=w1q[:, 0:DIM])
            nc.sync.dma_start(out=xsb[0][:, 0:4, :], in_=xP[:, 0:2048])
            nc.sync.dma_start(out=xsb[0][:, 4:8, :], in_=xP[:, 2048:4096])
            nc.sync.dma_start(out=w1qsb[:, 1:4, :], in_=w1q[:, DIM:4 * DIM])
            nc.sync.dma_start(out=w1ksb[:, :, :], in_=w1k[:, :])
            nc.sync.dma_start(
                out=xsb[1][:, :, :], in_=xP[:, 4096:2 * 4096]
            )
            nc.sync.dma_start(out=w1vsb[:, :, :], in_=w1v[:, :])
            nc.sync.dma_start(
                out=xsb[2][:, :, :], in_=xP[:, 2 * 4096:3 * 4096]
            )
            nc.sync.dma_start(out=w2sb[:, :, :], in_=w2[:, :])
            nc.sync.dma_start(out=ec_sb[:, :], in_=ec[:, :])
            for ib in range(3, NTB):
                nc.sync.dma_start(
                    out=xsb[ib][:, :, :],
                    in_=xP[:, ib * 4096:(ib + 1) * 4096],
                )

            phase_a(nc, tc, NTB, xsb, w1qsb, w1ksb, w1vsb, scratch,
                    qt, kvt, zt, ones_col)
            phase_b(nc, tc, NTB, w2sb, ec_sb, ones_col, y,
                    qt, kvt, zt, zcol, Zb, Ms)

    nc.compile()
    return nc


def phase_a(nc, tc, NTB, xsb, w1qsb, w1ksb, w1vsb, scratch,
            qt, kvt, zt, ones_col):
    with (
        tc.tile_pool(name="phA_sb", bufs=3) as pa,
        tc.tile_pool(name="proj_ps", bufs=6, space="PSUM") as proj_ps,
        tc.tile_pool(name="hold_ps", bufs=1, space="PSUM") as hold_ps,
    ):
        # PSUM accumulators held across all of phase A (one bank each).
        # kvps holds 4 interleaved accumulation regions; zero it up
        # front and accumulate with start=False everywhere (hardware
        # bank-clear on start would wipe sibling regions).
        kvps = hold_ps.tile([P, 4 * P], F32, name="kvps", tag="kvps")
        zps = hold_ps.tile([1, CH], F32, name="zps", tag="zps")
        nc.vector.memset(kvps[:, :], 0.0)

        # PE warmup: dummy matmuls on zeroed scratch keep the PE busy from
        # ~t=0.5us so the HAM clock-gate is at 8/8 when real data lands,
        # and fill the otherwise-idle DMA ramp.
        for w in range(WARMUP_MMS):
            wps = proj_ps.tile([P, 512], F32, name=f"warm_{w}", tag="proj")
            nc.tensor.matmul(
                wps[:, :], scratch[:, 0:P], scratch[:, :],
                start=True, stop=True,
            )

        def elu1(out_ap, src_ps, tag_sfx, nm):
            """out = elu(src)+1 = max(src + 1, exp(min(src, 0))), bf16 out."""
            m_ = pa.tile([P, 512], F32, name=f"m_{nm}", tag=f"m{tag_sfx}")
            e_ = pa.tile([P, 512], F32, name=f"e_{nm}", tag=f"e{tag_sfx}")
            nc.vector.tensor_scalar_min(m_[:, :], src_ps[:, :], 0.0)
            nc.scalar.activation(e_[:, :], m_[:, :], AF.Exp)
            nc.vector.scalar_tensor_tensor(
                out_ap, src_ps[:, :], 1.0, e_[:, :], ALU.add, ALU.max
            )

        def zkv_block(ib, ksb, vsb):
            """z/kv accumulation for block ib; deferred one block so the
            k/v eviction chains never stall the PE."""
            first = (ib == 0)
            last = (ib == NTB - 1)
            for t in range(4):
                csl_t = slice(t * P, (t + 1) * P)
                # z += ones^T k   [1, 512]
                nc.tensor.matmul(
                    zps[0:1, :], ones_col[:, :], ksb[t][:, :],
                    start=(first and t == 0), stop=(last and t == 3),
                    skip_group_check=True,
                )
                # kvT[j] += v_pair^T k_pair   [128, 128] per head-pair.
                # One accumulation group for the whole packed bank:
                # start=False everywhere, per-element has_written handles
                # first-write-overwrite onto the memset zeros.
                for j in range(4):
                    csl = slice(j * P, (j + 1) * P)
                    nc.tensor.matmul(
                        kvps[:, csl], vsb[t][:, csl], ksb[t][:, csl],
                        start=False, stop=(last and t == 3 and j == 3),
                        skip_group_check=True,
                    )

        # ---- phase A: projections + kv/z accumulation ----
        prev_kv = None
        for ib in range(NTB):
            tsl = slice(ib * 512, (ib + 1) * 512)

            # q projection (c-major) with elu+1, into persistent qt
            for j in range(4):
                qps = proj_ps.tile([P, 512], F32, name=f"qps_{ib}_{j}", tag="proj")
                for ct in range(8):
                    nc.tensor.matmul(
                        qps[:, :],
                        w1qsb[:, j:j + 1, ct * P:(ct + 1) * P],
                        xsb[ib][:, ct:ct + 1, :],
                        start=(ct == 0),
                        stop=(ct == 7),
                    )
                elu1(qt[j][:, tsl], qps, "q", f"q_{ib}_{j}")

            # z/kv for the PREVIOUS block: its k/v evictions finished
            # while this block's q matmuls were streaming.
            if prev_kv is not None:
                zkv_block(ib - 1, *prev_kv)

            # k, v projections (token-major) per 128-token block
            ksb, vsb = [], []
            for t in range(4):
                tok = slice(t * P, (t + 1) * P)
                kps = proj_ps.tile([P, 512], F32, name=f"kps_{ib}_{t}", tag="proj")
                for ct in range(8):
                    nc.tensor.matmul(
                        kps[:, :],
                        xsb[ib][:, ct:ct + 1, tok],
                        w1ksb[:, ct:ct + 1, :],
                        start=(ct == 0),
                        stop=(ct == 7),
                    )
                k_sb = pa.tile(
                    [P, 512], BF16, name=f"k_{ib}_{t}", tag="k_sb", bufs=9
                )
                elu1(k_sb[:, :], kps, "k", f"k_{ib}_{t}")
                ksb.append(k_sb)

                vps = proj_ps.tile([P, 512], F32, name=f"vps_{ib}_{t}", tag="proj")
                for ct in range(8):
                    nc.tensor.matmul(
                        vps[:, :],
                        xsb[ib][:, ct:ct + 1, tok],
                        w1vsb[:, ct:ct + 1, :],
                        start=(ct == 0),
                        stop=(ct == 7),
                    )
                v_sb = pa.tile(
                    [P, 512], BF16, name=f"v_{ib}_{t}", tag="v_sb", bufs=9
                )
                nc.scalar.copy(v_sb[:, :], vps[:, :])
                vsb.append(v_sb)
            prev_kv = (ksb, vsb)

        zkv_block(NTB - 1, *prev_kv)

        # ---- evict PSUM accumulators before releasing phase-A pools ----
        # kvt holds memset zeros; only the diagonal 64-blocks are written.
        # Alternate DVE/ACT so the boundary doesn't serialize on one queue.
        nc.vector.tensor_copy(zt[0:1, :], zps[0:1, :])
        for j in range(4):
            eng = nc.vector.tensor_copy if j % 2 == 0 else nc.scalar.copy
            eng(kvt[j][0:64, 0:64], kvps[0:64, j * P:j * P + 64])
            eng(
                kvt[j][64:128, 64:128],
                kvps[64:128, j * P + 64:(j + 1) * P],
            )


def phase_b(nc, tc, NTB, w2sb, ec_sb, ones_col, y,
            qt, kvt, zt, zcol, Zb, Ms):
    with (
        tc.tile_pool(name="phB_sb", bufs=2) as pb,
        tc.tile_pool(name="qsc_pool", bufs=8) as qp,
        tc.tile_pool(name="d_ps", bufs=2, space="PSUM") as d_ps,
        tc.tile_pool(name="bc_ps", bufs=2, space="PSUM") as bc_ps,
        tc.tile_pool(name="y_ps", bufs=4, space="PSUM") as y_ps,
    ):
        # ---- Zb first: z row -> per-partition columns via K=1 matmuls ----
        # zc[:, j] = zt[0, j*128:(j+1)*128]^T  (plain matmul with N=1)
        zc = bc_ps.tile([P, 512], F32, name="zc", tag="bc")
        nc.vector.memset(zc[:, 0:4], 0.0)
        for j in range(4):
            nc.tensor.matmul(
                zc[:, j:j + 1],
                zt[0:1, j * P:(j + 1) * P],
                ones_col[0:1, 0:1],
                start=False, stop=(j == 3), skip_group_check=True,
            )
        nc.vector.tensor_copy(zcol[:, :], zc[:, 0:4])
        for j in range(4):
            nc.vector.tensor_copy(
                Zb[j][0:64, 2 * j:2 * j + 1], zcol[0:64, j:j + 1]
            )
            nc.vector.tensor_copy(
                Zb[j][64:128, 2 * j + 1:2 * j + 2], zcol[64:128, j:j + 1]
            )

        def den_recip(ib):
            """den = Zb^T q -> r = 1/den (fast approx), bf16 rT."""
            tsl = slice(ib * 512, (ib + 1) * 512)
            dps = d_ps.tile([8, 512], F32, name=f"dps_{ib}", tag="d")
            for j in range(4):
                nc.tensor.matmul(
                    dps[:, :], Zb[j][:, :], qt[j][:, tsl],
                    start=(j == 0), stop=(j == 3),
                )
            rf = pb.tile([8, 512], F32, name=f"rf_{ib}", tag="rf")
            nc.vector.reciprocal_approx_fast(out=rf[:, :], in_=dps[:, :])
            rT = pb.tile([8, 512], BF16, name=f"rT_{ib}", tag="rT")
            nc.scalar.copy(rT[:, :], rf[:, :])
            return rT

        def bc_qsc(ib, rT):
            """rbc = E^T r broadcast; qsc = qt * rbc (bf16)."""
            tsl = slice(ib * 512, (ib + 1) * 512)
            qsc = []
            for j in range(4):
                bcp = bc_ps.tile([P, 512], F32, name=f"bcp_{ib}_{j}", tag="bc")
                nc.tensor.matmul(
                    bcp[:, :], ec_sb[:, j * P:(j + 1) * P], rT[:, :],
                    start=True, stop=True,
                )
                qs = qp.tile([P, 512], BF16, name=f"qsc_{ib}_{j}", tag="qsc")
                nc.vector.tensor_mul(qs[:, :], qt[j][:, tsl], bcp[:, :])
                qsc.append(qs)
            return qsc

        # M = kvT^T @ W2 per ch-tile: kvt-gated, runs while the Zb DVE
        # chain completes; then den/recip prologue (2 deep).
        for j in range(4):
            for h in range(2):
                hsl = slice(h * 512, (h + 1) * 512)
                mps = y_ps.tile([P, 512], F32, name=f"mps_{j}_{h}", tag="y")
                nc.tensor.matmul(
                    mps[:, :], kvt[j][:, :], w2sb[:, j:j + 1, hsl],
                    start=True, stop=True,
                )
                if h == 0:
                    nc.vector.tensor_copy(Ms[j][:, hsl], mps[:, :])
                else:
                    nc.scalar.copy(Ms[j][:, hsl], mps[:, :])

        rTs = {0: den_recip(0)}
        rTs[1] = den_recip(1)
        qscs = {0: bc_qsc(0, rTs.pop(0))}

        # ---- fused main loop: den(ib+2) / bc+qsc(ib+1) / y(ib) ----
        for ib in range(NTB):
            if ib + 2 < NTB:
                rTs[ib + 2] = den_recip(ib + 2)
            if ib + 1 < NTB:
                qscs[ib + 1] = bc_qsc(ib + 1, rTs.pop(ib + 1))

            qsc = qscs.pop(ib)
            for t in range(4):
                tok = slice(t * P, (t + 1) * P)
                row = (ib * 4 + t) * P
                yp = [
                    y_ps.tile([P, 512], F32, name=f"yps_{ib}_{t}_{h}", tag="y")
                    for h in range(2)
                ]
                for h in range(2):
                    hsl = slice(h * 512, (h + 1) * 512)
                    for j in range(4):
                        nc.tensor.matmul(
                            yp[h][:, :], qsc[j][:, tok], Ms[j][:, hsl],
                            start=(j == 0), stop=(j == 3),
                        )
                y_sb = pb.tile(
                    [P, DIM], F32, name=f"y_{ib}_{t}", tag="y_sb", bufs=3
                )
                nc.vector.tensor_copy(y_sb[:, 0:512], yp[0][:, :])
                nc.scalar.copy(y_sb[:, 512:1024], yp[1][:, :])
                nc.sync.dma_start(out=y[row:row + P, :], in_=y_sb[:, :])


_NC_CACHE = {}


def _get_nc(T=T_FULL):
    if T not in _NC_CACHE:
        _NC_CACHE[T] = build_nc(T)
    return _NC_CACHE[T]


def make_in_maps(x, W_qkv, W_out, b_out):
    import ml_dtypes

    bf16 = ml_dtypes.bfloat16
    x = np.asarray(x, dtype=np.float32)
    W_qkv = np.asarray(W_qkv, dtype=np.float32).astype(bf16)
    W_out = np.asarray(W_out, dtype=np.float32).astype(bf16)
    NTB = T_FULL // 512

    # xP[p, ib*4096 + ct*512 + tl] = x[b][ib*512 + tl, ct*128 + p]
    xPs = []
    for b in range(B):
        a = x[b].astype(bf16).reshape(NTB, 512, 8, P)
        xPs.append(
            np.ascontiguousarray(
                a.transpose(3, 0, 2, 1).reshape(P, NTB * 8 * 512)
            )
        )

    w1qs, w1ks, w1vs, w2s = [], [], [], []
    for hg in range(2):
        cs = slice(hg * CH, (hg + 1) * CH)
        Wq = W_qkv[:, cs]                                   # [1024, 512]
        Wk = W_qkv[:, DIM + hg * CH:DIM + (hg + 1) * CH]
        Wv = W_qkv[:, 2 * DIM + hg * CH:2 * DIM + (hg + 1) * CH]
        # w1q[p, j*1024 + ct*128 + c] = Wq[ct*128 + p, j*128 + c]
        w1qs.append(
            np.ascontiguousarray(
                Wq.reshape(8, P, 4, P).transpose(1, 2, 0, 3).reshape(P, 4 * DIM)
            )
        )
        # w1k[p, ct*512 + n] = Wk[ct*128 + p, n]
        w1ks.append(
            np.ascontiguousarray(
                Wk.reshape(8, P, CH).transpose(1, 0, 2).reshape(P, 8 * CH)
            )
        )
        w1vs.append(
            np.ascontiguousarray(
                Wv.reshape(8, P, CH).transpose(1, 0, 2).reshape(P, 8 * CH)
            )
        )
        # w2[p, j*1024 + n] = W_out[hg*512 + j*128 + p, n]
        w2s.append(
            np.ascontiguousarray(
                W_out[cs, :].reshape(4, P, DIM).transpose(1, 0, 2).reshape(P, 4 * DIM)
            )
        )
    ecm = make_ec().astype(bf16)

    in_maps = []
    for core in range(N_CORES):
        b, hg = core // 2, core % 2
        in_maps.append({
            "xP": xPs[b], "w1q": w1qs[hg], "w1k": w1ks[hg],
            "w1v": w1vs[hg], "w2": w2s[hg], "ec": ecm,
        })
    return in_maps


def make_ec():
    """E selector: ec[h, j*128+p] = 1 iff head-of-partition-p-in-tile-j == h."""
    ecm = np.zeros((8, CH), dtype=np.float32)
    for j in range(4):
        ecm[2 * j, j * P:j * P + 64] = 1.0
        ecm[2 * j + 1, j * P + 64:(j + 1) * P] = 1.0
    return ecm


def kernel(x, W_qkv, W_out, b_out):
    from concourse.bass_utils import run_bass_kernel_spmd

    nc = _get_nc(T_FULL)
    in_maps = make_in_maps(x, W_qkv, W_out, b_out)
    res = run_bass_kernel_spmd(nc, in_maps, core_ids=list(range(N_CORES))).results
    bo = np.asarray(b_out, dtype=np.float32)
    out = np.empty((B, T_FULL, DIM), dtype=np.float32)
    for b in range(B):
        out[b] = res[2 * b]["y"] + res[2 * b + 1]["y"] + bo
    return out


# revision 32
# speedup vs baseline: 1.2043x; 1.0148x over previous
"""Linear attention kernel for 8 Trainium2 NeuronCores.

Sharding: core = 2*b + hg  (b in 0..3 batches, hg in 0..1 head-groups of 8 heads).
Fully data-parallel - no collectives; host sums the two head-group partials per
batch. Each core adds bias/2 so the pair-sum carries the full bias.

Per-core math (T=4096 tokens, CH=512 = 8 heads x 64, DIM=1024):
  qT   = (x @ Wq)^T          c-major [CH, T], elu+1
  k,v  = x @ Wk, x @ Wv      token-major [T, CH], elu+1 on k
  kvT  = v^T k  (per head-pair, diagonal 64-blocks valid), accumulated in PSUM
  z    = ones^T k            [1, CH], accumulated in PSUM
  den  = Zblock^T qT         [8, T]   (Zblock = block-diag scatter of z)
  r    = 1/den;  rbc = E^T r (broadcast r over each head's 64 rows)
  qsc  = qT * rbc
  M    = kvT^T @ W2  (per 128-row ch-tile; off-diag blocks of kvT zeroed)
  y    = qsc^T @ M + bias/2  token-major [T, DIM]

v2 scheduling: batched 3D-AP DMAs ordered by first use, PE warmup matmuls
during the DMA ramp (HAM stays warm), 3-op elu+1 (min/exp/stt-max), z
transposed to columns via K=1 matmuls at the phase boundary (no SB->SB DMA),
and a fused phase B with 2-stage lookahead so den/bc/recip of ib+1/ib+2
overlap the y matmuls of ib and the y writeback DMA is spread evenly.
"""

import sys

sys.path.insert(0, "/opt/trn_rl_repo")

import numpy as np

import concourse.bass as bass
import concourse.mybir as mybir
import concourse.tile as tile
from concourse import bacc

F32 = mybir.dt.float32
BF16 = mybir.dt.bfloat16
AF = mybir.ActivationFunctionType
ALU = mybir.AluOpType

DIM = 1024      # model dim (contraction for projections)
CH = 512        # per-core channels (8 heads x 64)
P = 128

N_CORES = 8
B, T_FULL = 4, 4096

WARMUP_MMS = 12


def build_nc(T=T_FULL):
    NTB = T // 512          # 512-token blocks
    nc = bacc.Bacc(None, target_bir_lowering=False, debug=False)

    # All inputs host-repacked partition-major so every DMA line is a
    # contiguous 2-8KB row segment (packet-count, not bandwidth, limits
    # the startup ramp).
    xP = nc.declare_dram_parameter("xP", [P, NTB * 8 * 512], BF16, isOutput=False)
    w1q = nc.declare_dram_parameter("w1q", [P, 4 * 1024], BF16, isOutput=False)
    w1k = nc.declare_dram_parameter("w1k", [P, 8 * 512], BF16, isOutput=False)
    w1v = nc.declare_dram_parameter("w1v", [P, 8 * 512], BF16, isOutput=False)
    w2 = nc.declare_dram_parameter("w2", [P, 4 * 1024], BF16, isOutput=False)
    ec = nc.declare_dram_parameter("ec", [8, CH], BF16, isOutput=False)
    y = nc.declare_dram_parameter("y", [T, DIM], F32, isOutput=True)

    with tile.TileContext(nc) as tc:
        with tc.tile_pool(name="persist", bufs=1) as pp:
            # ---- persistent tiles ----
            scratch = pp.tile([P, 512], BF16, name="scratch", tag="scratch")
            ones_col = pp.tile([P, 1], BF16, name="ones_col", tag="ones_col")
            w1qsb = pp.tile([P, 4, DIM], BF16, name="w1qsb", tag="w1qsb")
            w1ksb = pp.tile([P, 8, CH], BF16, name="w1ksb", tag="w1ksb")
            w1vsb = pp.tile([P, 8, CH], BF16, name="w1vsb", tag="w1vsb")
            w2sb = pp.tile([P, 4, DIM], BF16, name="w2sb", tag="w2sb")
            ec_sb = pp.tile([8, CH], BF16, name="ec_sb", tag="ec_sb")
            xsb = [
                pp.tile([P, 8, 512], BF16, name=f"xsb_{ib}", tag=f"xsb_{ib}")
                for ib in range(NTB)
            ]
            qt = [
                pp.tile([P, T], BF16, name=f"qt_{j}", tag=f"qt_{j}")
                for j in range(4)
            ]
            kvt = [
                pp.tile([P, P], BF16, name=f"kvt_{j}", tag=f"kvt_{j}")
                for j in range(4)
            ]
            Zb = [
                pp.tile([P, 8], BF16, name=f"Zb_{j}", tag=f"Zb_{j}")
                for j in range(4)
            ]
            Ms = [
                pp.tile([P, DIM], BF16, name=f"Ms_{j}", tag=f"Ms_{j}")
                for j in range(4)
            ]
            zt = pp.tile([1, CH], BF16, name="zt", tag="zt")
            zcol = pp.tile([P, 4], F32, name="zcol", tag="zcol")

            # memsets first: scratch gates the PE warmup, kvt/Zb hold zeros
            # that the boundary only partially overwrites.
            nc.vector.memset(scratch[:, :], 0.0)
            nc.vector.memset(ones_col[:, :], 1.0)
            for j in range(4):
                nc.vector.memset(kvt[j][:, :], 0.0)
                nc.vector.memset(Zb[j][:, :], 0.0)

            # ---- DMAs, ordered by first use (each splits over 16 queues) ----
            nc.sync.dma_start(out=w1qsb[:, 0:1, :], in_=w1q[:, 0:DIM])
            nc.sync.dma_start(out=xsb[0][:, 0:4, :], in_=xP[:, 0:2048])
            nc.sync.dma_start(out=xsb[0][:, 4:8, :], in_=xP[:, 2048:4096])
            nc.sync.dma_start(out=w1qsb[:, 1:4, :], in_=w1q[:, DIM:4 * DIM])
            nc.sync.dma_start(out=w1ksb[:, :, :], in_=w1k[:, :])
            nc.sync.dma_start(out=w1vsb[:, :, :], in_=w1v[:, :])
            nc.sync.dma_start(
                out=xsb[1][:, :, :], in_=xP[:, 4096:2 * 4096]
            )
            nc.sync.dma_start(
                out=xsb[2][:, :, :], in_=xP[:, 2 * 4096:3 * 4096]
            )
            nc.sync.dma_start(out=w2sb[:, :, :], in_=w2[:, :])
            nc.sync.dma_start(out=ec_sb[:, :], in_=ec[:, :])
            for ib in range(3, NTB):
                nc.sync.dma_start(
                    out=xsb[ib][:, :, :],
                    in_=xP[:, ib * 4096:(ib + 1) * 4096],
                )

            phase_a(nc, tc, NTB, xsb, w1qsb, w1ksb, w1vsb, scratch,
                    qt, kvt, zt, ones_col)
            phase_b(nc, tc, NTB, w2sb, ec_sb, ones_col, y,
                    qt, kvt, zt, zcol, Zb, Ms)

    nc.compile()
    return nc


def phase_a(nc, tc, NTB, xsb, w1qsb, w1ksb, w1vsb, scratch,
            qt, kvt, zt, ones_col):
    with (
        tc.tile_pool(name="phA_sb", bufs=3) as pa,
        tc.tile_pool(name="proj_ps", bufs=6, space="PSUM") as proj_ps,
        tc.tile_pool(name="hold_ps", bufs=1, space="PSUM") as hold_ps,
    ):
        # PSUM accumulators held across all of phase A (one bank each).
        # kvps holds 4 interleaved accumulation regions; zero it up
        # front and accumulate with start=False everywhere (hardware
        # bank-clear on start would wipe sibling regions).
        kvps = hold_ps.tile([P, 4 * P], F32, name="kvps", tag="kvps")
        zps = hold_ps.tile([1, CH], F32, name="zps", tag="zps")
        nc.vector.memset(kvps[:, :], 0.0)

        # PE warmup: dummy matmuls on zeroed scratch keep the PE busy from
        # ~t=0.5us so the HAM clock-gate is at 8/8 when real data lands,
        # and fill the otherwise-idle DMA ramp.
        for w in range(WARMUP_MMS):
            wps = proj_ps.tile([P, 512], F32, name=f"warm_{w}", tag="proj")
            nc.tensor.matmul(
                wps[:, :], scratch[:, 0:P], scratch[:, :],
                start=True, stop=True,
            )

        def elu1(out_ap, src_ps, tag_sfx, nm):
            """out = elu(src)+1 = max(src + 1, exp(min(src, 0))), bf16 out."""
            m_ = pa.tile([P, 512], F32, name=f"m_{nm}", tag=f"m{tag_sfx}")
            e_ = pa.tile([P, 512], F32, name=f"e_{nm}", tag=f"e{tag_sfx}")
            nc.vector.tensor_scalar_min(m_[:, :], src_ps[:, :], 0.0)
            nc.scalar.activation(e_[:, :], m_[:, :], AF.Exp)
            nc.vector.scalar_tensor_tensor(
                out_ap, src_ps[:, :], 1.0, e_[:, :], ALU.add, ALU.max
            )

        def zkv_block(ib, ksb, vsb):
            """z/kv accumulation for block ib; deferred one block so the
            k/v eviction chains never stall the PE."""
            first = (ib == 0)
            last = (ib == NTB - 1)
            for t in range(4):
                csl_t = slice(t * P, (t + 1) * P)
                # z += ones^T k   [1, 512]
                nc.tensor.matmul(
                    zps[0:1, :], ones_col[:, :], ksb[t][:, :],
                    start=(first and t == 0), stop=(last and t == 3),
                    skip_group_check=True,
                )
                # kvT[j] += v_pair^T k_pair   [128, 128] per head-pair.
                # One accumulation group for the whole packed bank:
                # start=False everywhere, per-element has_written handles
                # first-write-overwrite onto the memset zeros.
                for j in range(4):
                    csl = slice(j * P, (j + 1) * P)
                    nc.tensor.matmul(
                        kvps[:, csl], vsb[t][:, csl], ksb[t][:, csl],
                        start=False, stop=(last and t == 3 and j == 3),
                        skip_group_check=True,
                    )

        # ---- phase A: projections + kv/z accumulation ----
        prev_kv = None
        for ib in range(NTB):
            tsl = slice(ib * 512, (ib + 1) * 512)

            # q projection (c-major) with elu+1, into persistent qt
            for j in range(4):
                qps = proj_ps.tile([P, 512], F32, name=f"qps_{ib}_{j}", tag="proj")
                for ct in range(8):
                    nc.tensor.matmul(
                        qps[:, :],
                        w1qsb[:, j:j + 1, ct * P:(ct + 1) * P],
                        xsb[ib][:, ct:ct + 1, :],
                        start=(ct == 0),
                        stop=(ct == 7),
                    )
                elu1(qt[j][:, tsl], qps, "q", f"q_{ib}_{j}")

            # z/kv for the PREVIOUS block: its k/v evictions finished
            # while this block's q matmuls were streaming.
            if prev_kv is not None:
                zkv_block(ib - 1, *prev_kv)

            # k, v projections (token-major) per 128-token block
            ksb, vsb = [], []
            for t in range(4):
                tok = slice(t * P, (t + 1) * P)
                kps = proj_ps.tile([P, 512], F32, name=f"kps_{ib}_{t}", tag="proj")
                for ct in range(8):
                    nc.tensor.matmul(
                        kps[:, :],
                        xsb[ib][:, ct:ct + 1, tok],
                        w1ksb[:, ct:ct + 1, :],
                        start=(ct == 0),
                        stop=(ct == 7),
                    )
                k_sb = pa.tile(
                    [P, 512], BF16, name=f"k_{ib}_{t}", tag="k_sb", bufs=9
                )
                elu1(k_sb[:, :], kps, "k", f"k_{ib}_{t}")
                ksb.append(k_sb)

                vps = proj_ps.tile([P, 512], F32, name=f"vps_{ib}_{t}", tag="proj")
                for ct in range(8):
                    nc.tensor.matmul(
                        vps[:, :],
                        xsb[ib][:, ct:ct + 1, tok],
                        w1vsb[:, ct:ct + 1, :],
                        start=(ct == 0),
                        stop=(ct == 7),
                    )
                v_sb = pa.tile(
                    [P, 512], BF16, name=f"v_{ib}_{t}", tag="v_sb", bufs=9
                )
                nc.scalar.copy(v_sb[:, :], vps[:, :])
                vsb.append(v_sb)
            prev_kv = (ksb, vsb)

        zkv_block(NTB - 1, *prev_kv)

        # ---- evict PSUM accumulators before releasing phase-A pools ----
        # kvt holds memset zeros; only the diagonal 64-blocks are written.
        # Alternate DVE/ACT so the boundary doesn't serialize on one queue.
        nc.vector.tensor_copy(zt[0:1, :], zps[0:1, :])
        for j in range(4):
            eng = nc.vector.tensor_copy if j % 2 == 0 else nc.scalar.copy
            eng(kvt[j][0:64, 0:64], kvps[0:64, j * P:j * P + 64])
            eng(
                kvt[j][64:128, 64:128],
                kvps[64:128, j * P + 64:(j + 1) * P],
            )


def phase_b(nc, tc, NTB, w2sb, ec_sb, ones_col, y,
            qt, kvt, zt, zcol, Zb, Ms):
    with (
        tc.tile_pool(name="phB_sb", bufs=2) as pb,
        tc.tile_pool(name="qsc_pool", bufs=8) as qp,
        tc.tile_pool(name="d_ps", bufs=2, space="PSUM") as d_ps,
        tc.tile_pool(name="bc_ps", bufs=2, space="PSUM") as bc_ps,
        tc.tile_pool(name="y_ps", bufs=4, space="PSUM") as y_ps,
    ):
        # ---- Zb first: z row -> per-partition columns via K=1 matmuls ----
        # zc[:, j] = zt[0, j*128:(j+1)*128]^T  (plain matmul with N=1)
        zc = bc_ps.tile([P, 512], F32, name="zc", tag="bc")
        nc.vector.memset(zc[:, 0:4], 0.0)
        for j in range(4):
            nc.tensor.matmul(
                zc[:, j:j + 1],
                zt[0:1, j * P:(j + 1) * P],
                ones_col[0:1, 0:1],
                start=False, stop=(j == 3), skip_group_check=True,
            )
        nc.vector.tensor_copy(zcol[:, :], zc[:, 0:4])
        for j in range(4):
            nc.vector.tensor_copy(
                Zb[j][0:64, 2 * j:2 * j + 1], zcol[0:64, j:j + 1]
            )
            nc.vector.tensor_copy(
                Zb[j][64:128, 2 * j + 1:2 * j + 2], zcol[64:128, j:j + 1]
            )

        def den_recip(ib):
            """den = Zb^T q -> r = 1/den (fast approx), bf16 rT."""
            tsl = slice(ib * 512, (ib + 1) * 512)
            dps = d_ps.tile([8, 512], F32, name=f"dps_{ib}", tag="d")
            for j in range(4):
                nc.tensor.matmul(
                    dps[:, :], Zb[j][:, :], qt[j][:, tsl],
                    start=(j == 0), stop=(j == 3),
                )
            rf = pb.tile([8, 512], F32, name=f"rf_{ib}", tag="rf")
            nc.vector.reciprocal_approx_fast(out=rf[:, :], in_=dps[:, :])
            rT = pb.tile([8, 512], BF16, name=f"rT_{ib}", tag="rT")
            nc.scalar.copy(rT[:, :], rf[:, :])
            return rT

        def bc_qsc(ib, rT):
            """rbc = E^T r broadcast; qsc = qt * rbc (bf16)."""
            tsl = slice(ib * 512, (ib + 1) * 512)
            qsc = []
            for j in range(4):
                bcp = bc_ps.tile([P, 512], F32, name=f"bcp_{ib}_{j}", tag="bc")
                nc.tensor.matmul(
                    bcp[:, :], ec_sb[:, j * P:(j + 1) * P], rT[:, :],
                    start=True, stop=True,
                )
                qs = qp.tile([P, 512], BF16, name=f"qsc_{ib}_{j}", tag="qsc")
                nc.vector.tensor_mul(qs[:, :], qt[j][:, tsl], bcp[:, :])
                qsc.append(qs)
            return qsc

        # M = kvT^T @ W2 per ch-tile: kvt-gated, runs while the Zb DVE
        # chain completes; then den/recip prologue (2 deep).
        for j in range(4):
            for h in range(2):
                hsl = slice(h * 512, (h + 1) * 512)
                mps = y_ps.tile([P, 512], F32, name=f"mps_{j}_{h}", tag="y")
                nc.tensor.matmul(
                    mps[:, :], kvt[j][:, :], w2sb[:, j:j + 1, hsl],
                    start=True, stop=True,
                )
                if h == 0:
                    nc.vector.tensor_copy(Ms[j][:, hsl], mps[:, :])
                else:
                    nc.scalar.copy(Ms[j][:, hsl], mps[:, :])

        rTs = {0: den_recip(0)}
        rTs[1] = den_recip(1)
        qscs = {0: bc_qsc(0, rTs.pop(0))}

        # ---- fused main loop: den(ib+2) / bc+qsc(ib+1) / y(ib) ----
        for ib in range(NTB):
            if ib + 2 < NTB:
                rTs[ib + 2] = den_recip(ib + 2)
            if ib + 1 < NTB:
                qscs[ib + 1] = bc_qsc(ib + 1, rTs.pop(ib + 1))

            qsc = qscs.pop(ib)
            for t in range(4):
                tok = slice(t * P, (t + 1) * P)
                row = (ib * 4 + t) * P
                yp = [
                    y_ps.tile([P, 512], F32, name=f"yps_{ib}_{t}_{h}", tag="y")
                    for h in range(2)
                ]
                for h in range(2):
                    hsl = slice(h * 512, (h + 1) * 512)
                    for j in range(4):
                        nc.tensor.matmul(
                            yp[h][:, :], qsc[j][:, tok], Ms[j][:, hsl],
                            start=(j == 0), stop=(j == 3),
                        )
                y_sb = pb.tile(
                    [P, DIM], F32, name=f"y_{ib}_{t}", tag="y_sb", bufs=3
                )
                nc.vector.tensor_copy(y_sb[:, 0:512], yp[0][:, :])
                nc.scalar.copy(y_sb[:, 512:1024], yp[1][:, :])
                if ib == NTB - 1 and t == 3:
                    # final tile: ship the h0 half while h1 still evicts,
                    # so the tail chain is only the h1 copy + 256KB DMA
                    nc.sync.dma_start(
                        out=y[row:row + P, 0:512], in_=y_sb[:, 0:512]
                    )
                    nc.sync.dma_start(
                        out=y[row:row + P, 512:1024], in_=y_sb[:, 512:1024]
                    )
                else:
                    nc.sync.dma_start(out=y[row:row + P, :], in_=y_sb[:, :])


_NC_CACHE = {}


def _get_nc(T=T_FULL):
    if T not in _NC_CACHE:
        _NC_CACHE[T] = build_nc(T)
    return _NC_CACHE[T]


def make_in_maps(x, W_qkv, W_out, b_out):
    import ml_dtypes

    bf16 = ml_dtypes.bfloat16
    x = np.asarray(x, dtype=np.float32)
    W_qkv = np.asarray(W_qkv, dtype=np.float32).astype(bf16)
    W_out = np.asarray(W_out, dtype=np.float32).astype(bf16)
    NTB = T_FULL // 512

    # xP[p, ib*4096 + ct*512 + tl] = x[b][ib*512 + tl, ct*128 + p]
    xPs = []
    for b in range(B):
        a = x[b].astype(bf16).reshape(NTB, 512, 8, P)
        xPs.append(
            np.ascontiguousarray(
                a.transpose(3, 0, 2, 1).reshape(P, NTB * 8 * 512)
            )
        )

    w1qs, w1ks, w1vs, w2s = [], [], [], []
    for hg in range(2):
        cs = slice(hg * CH, (hg + 1) * CH)
        Wq = W_qkv[:, cs]                                   # [1024, 512]
        Wk = W_qkv[:, DIM + hg * CH:DIM + (hg + 1) * CH]
        Wv = W_qkv[:, 2 * DIM + hg * CH:2 * DIM + (hg + 1) * CH]
        # w1q[p, j*1024 + ct*128 + c] = Wq[ct*128 + p, j*128 + c]
        w1qs.append(
            np.ascontiguousarray(
                Wq.reshape(8, P, 4, P).transpose(1, 2, 0, 3).reshape(P, 4 * DIM)
            )
        )
        # w1k[p, ct*512 + n] = Wk[ct*128 + p, n]
        w1ks.append(
            np.ascontiguousarray(
                Wk.reshape(8, P, CH).transpose(1, 0, 2).reshape(P, 8 * CH)
            )
        )
        w1vs.append(
            np.ascontiguousarray(
                Wv.reshape(8, P, CH).transpose(1, 0, 2).reshape(P, 8 * CH)
            )
        )
        # w2[p, j*1024 + n] = W_out[hg*512 + j*128 + p, n]
        w2s.append(
            np.ascontiguousarray(
                W_out[cs, :].reshape(4, P, DIM).transpose(1, 0, 2).reshape(P, 4 * DIM)
            )
        )
    ecm = make_ec().astype(bf16)

    in_maps = []
    for core in range(N_CORES):
        b, hg = core // 2, core % 2
        in_maps.append({
            "xP": xPs[b], "w1q": w1qs[hg], "w1k": w1ks[hg],
            "w1v": w1vs[hg], "w2": w2s[hg], "ec": ecm,
        })
    return in_maps


def make_ec():
    """E selector: ec[h, j*128+p] = 1 iff head-of-partition-p-in-tile-j == h."""
    ecm = np.zeros((8, CH), dtype=np.float32)
    for j in range(4):
        ecm[2 * j, j * P:j * P + 64] = 1.0
        ecm[2 * j + 1, j * P + 64:(j + 1) * P] = 1.0
    return ecm


def kernel(x, W_qkv, W_out, b_out):
    from concourse.bass_utils import run_bass_kernel_spmd

    nc = _get_nc(T_FULL)
    in_maps = make_in_maps(x, W_qkv, W_out, b_out)
    res = run_bass_kernel_spmd(nc, in_maps, core_ids=list(range(N_CORES))).results
    bo = np.asarray(b_out, dtype=np.float32)
    out = np.empty((B, T_FULL, DIM), dtype=np.float32)
    for b in range(B):
        out[b] = res[2 * b]["y"] + res[2 * b + 1]["y"] + bo
    return out
